# revision 4
# baseline (speedup 1.0000x reference)
"""Trainium2 Bass kernel for nn_Attention_44564580663760 — v4.

Single-head "attention" (B=8, S=2048, D=1024, fp32) with the reference's
quirk reproduced: scores = q @ v^T (k projection unused), causal mask,
softmax, ctx @ v, output projection.

Sharding: data-parallel — one batch element per NeuronCore (8 cores).

Changes vs the original baseline (~420us cold -> ~393us expected):
  - qT kept fully SBUF-resident (no DRAM round trip, no per-block reload)
  - vT projected first (all e-tiles), then qT: the 16 XBAR transpose DMAs
    that build natural-layout v overlap the qT projection on PE
  - l-sum matmuls only accumulate valid (causal) eT tiles -> the ragged
    memsets are gone
  - attention reads qT slices directly (no staging copies)
  - batched weight/bias/const loads; shared fine-grained projection psum
    pool so phase seams don't stall on whole-pool frees
  - bf16 output path: 1/l scale on ACT, bias-add on DVE at 2x 16-bit
    rate, bf16 stores, per-half store chain to shorten the tail
  - activation-function table preloaded during the head DMA bubble

Per-core dataflow (layouts chosen so the attention matrix never needs an
on-chip transpose; matmul contracts the partition dim):
    xT  [d, s]   (host-transposed input, bf16)
    vT  [e, s] = WvT.T @ xT (+bv)         -> resident + DRAM copy
    v   [k, d] = XBAR-transpose(vT)       -> resident
    qT  [e, s] = WqT.T @ xT (+bq)         -> resident
    per q-block (512 cols):
      scoresT [k, q] = vT.T @ qT          (fp32 psum)
      eT = exp(scoresT/32) (bf16), causal-masked
      ctxT [d, q]: lhsT = v slices, rhs = eT
      l[q] = column sums of eT via N=1 matmuls with ones (lhsT = eT)
      out [q, e]: lhsT = ctxT slices, rhs = WoT; scaled by 1/l, + bo
"""

import sys

sys.path.insert(0, "/opt/trn_rl_repo")

import contextlib

import numpy as np

import concourse.bacc as bacc
import concourse.bass as bass
import concourse.mybir as mybir
import concourse.tile as tile
from concourse.bass_utils import run_bass_kernel_spmd
from concourse.masks import make_identity

FP32 = mybir.dt.float32
BF16 = mybir.dt.bfloat16

B, S, D = 8, 2048, 1024
PT = 128
NTS = S // PT  # 16 s-tiles
NTD = D // PT  # 8 d-tiles
QB = 512  # q-block width
NBLK = S // QB  # 4
KPB = QB // PT  # 4 k-tiles per block
SCALE = 1.0 / np.sqrt(np.float32(D))
ACT = mybir.ActivationFunctionType


def build_nc(causal: bool, reps: int = 0) -> bass.Bass:
    nc = bacc.Bacc("TRN2", target_bir_lowering=False, debug=False)
    dram = {
        "xT": nc.declare_dram_parameter("xT", [D, S], BF16, isOutput=False),
        "WvT": nc.declare_dram_parameter("WvT", [D, D], BF16, isOutput=False),
        # batched params: weights tiled so one DMA fills 8 SBUF tiles
        "WqTt": nc.declare_dram_parameter("WqTt", [PT, NTD * D], BF16, isOutput=False),
        "WoTt": nc.declare_dram_parameter("WoTt", [PT, NTD * D], BF16, isOutput=False),
        "bqv": nc.declare_dram_parameter("bqv", [PT, 2 * NTD], FP32, isOutput=False),
        "bob": nc.declare_dram_parameter("bob", [PT, D], BF16, isOutput=False),
        "onut": nc.declare_dram_parameter("onut", [PT, 1 + PT], BF16, isOutput=False),
        "out": nc.declare_dram_parameter("out", [S, D], BF16, isOutput=True),
    }

    with (
        tile.TileContext(nc) as tc,
        tc.tile_pool(name="const", bufs=1) as constp,
        tc.tile_pool(name="dramp", bufs=1, space="DRAM") as dramp,
        tc.tile_pool(name="vTp", bufs=1) as vTp,
        tc.tile_pool(name="qTp", bufs=1) as qTp,
    ):
        vT_s = dramp.tile([D, S], BF16, tag="vts", name="vts")
        vT_t = [
            vTp.tile([PT, S], BF16, tag=f"vT{e}", name=f"vT{e}") for e in range(NTD)
        ]
        qT_t = [
            qTp.tile([PT, S], BF16, tag=f"qT{e}", name=f"qT{e}") for e in range(NTD)
        ]
        loop_ctx = tc.For_i(0, reps, 1) if reps else contextlib.nullcontext()
        with loop_ctx:
            _body(nc, tc, causal, constp, dram, vT_s, vT_t, qT_t)
    _dedup_ldweights(nc)
    nc.finalize()
    return nc


def _dedup_ldweights(nc):
    """Drop InstLdweights whose stationary operand matches the previous PE
    weight load (no intervening PE weight change) — the paired matmuls then
    reuse the already-loaded weights. Deps of a dropped LDW move to the next
    kept instruction so semaphore generation still orders correctly."""
    removed = {}
    n_drop = 0
    for bb in nc.main_func.blocks:
        insts = bb.instructions
        keep = []
        last_sig = None
        pending = []
        for ins in insts:
            drop = False
            if isinstance(ins, mybir.InstLdweights):
                sig = (
                    str(ins.ins[0]),
                    bool(ins.is_transpose),
                    str(ins.perf_mode),
                    str(ins.tile_position),
                )
                if sig == last_sig:
                    drop = True
                else:
                    last_sig = sig
            elif (
                getattr(ins, "engine", None) == mybir.EngineType.PE
                and isinstance(ins, mybir.InstMatmult)
                and ins.is_transpose
            ):
                # transpose-mode matmuls change the loaded weights
                last_sig = None
            if drop:
                pending.append(ins)
                n_drop += 1
                continue
            for p in pending:
                ins.merge_dependencies_from(p)
                removed[p.name] = ins.name
            pending = []
            keep.append(ins)
        assert not pending
        if len(keep) != len(insts):
            insts[:] = keep
    if removed:
        for bb in nc.main_func.blocks:
            for ins in bb.instructions:
                ins.remap_dependency_names(removed)
        if hasattr(nc, "inst_map"):
            for name in removed:
                nc.inst_map.pop(name, None)


def _body(nc, tc, causal, constp, dram, vT_s, vT_t, qT_t):
    with tc.tile_pool(name="xTqp", bufs=1) as xTqp:
        # ---- Phase A: vT projection (Wv pool scoped so it frees early) ----
        # xT/Wv stay per-tile DMAs (they gate the pipeline head); first tiles
        # are split so the very first matmul's operands arrive fast.
        xT_t = []

        # shared psum pool for both projection phases ([PT,1024] tiles,
        # 4 banks total) so neither phase seam waits on whole-pool frees
        # preload the activation function table while the head DMAs land
        with tc.tile_pool(name="atp", bufs=1) as atp:
            at_in = atp.tile([PT, 1], FP32, tag="ati", name="ati")
            at_out = atp.tile([PT, 1], FP32, tag="ato", name="ato")
            nc.gpsimd.memset(at_in[:], 0.0)
            nc.scalar.activation(at_out[:], at_in[:], ACT.Exp, scale=1.0)

        def project(psPp, W_t, b_t, out_t, store):
            for e in range(NTD):
                for h in range(2):
                    ps = psPp.tile([PT, 1024], FP32, tag="ps", name="ps")
                    for k in range(NTD):
                        for sc in range(2):
                            c0 = h * 1024 + sc * 512
                            nc.tensor.matmul(
                                ps[:, sc * 512 : (sc + 1) * 512],
                                W_t[k][:, e * PT : (e + 1) * PT],
                                xT_t[k][:, c0 : c0 + 512],
                                start=(k == 0),
                                stop=(k == NTD - 1),
                            )
                    nc.scalar.activation(
                        out_t[e][:, h * 1024 : (h + 1) * 1024],
                        ps[:],
                        ACT.Identity,
                        bias=b_t[e][:],
                        scale=1.0,
                    )
                if store:
                    nc.sync.dma_start(vT_s[e * PT : (e + 1) * PT, :], out_t[e][:])

        psP_ctx = tc.tile_pool(name="psP", bufs=2, space="PSUM")
        psPp = psP_ctx.__enter__()
        with tc.tile_pool(name="wvp", bufs=1) as wvp:
            Wv_t = []
            for k in range(NTD):
                xt = xTqp.tile([PT, S], BF16, tag=f"xT{k}", name=f"xT{k}")
                wv = wvp.tile([PT, D], BF16, tag=f"wv{k}", name=f"wv{k}")
                if k == 0:
                    nc.sync.dma_start(wv[:, 0:PT], dram["WvT"][0:PT, 0:PT])
                    nc.sync.dma_start(xt[:, 0:512], dram["xT"][0:PT, 0:512])
                    nc.sync.dma_start(wv[:, PT:D], dram["WvT"][0:PT, PT:D])
                    nc.sync.dma_start(xt[:, 512:S], dram["xT"][0:PT, 512:S])
                else:
                    nc.sync.dma_start(xt[:], dram["xT"][k * PT : (k + 1) * PT, :])
                    nc.sync.dma_start(wv[:], dram["WvT"][k * PT : (k + 1) * PT, :])
                xT_t.append(xt)
                Wv_t.append(wv)
            bqv_t = constp.tile([PT, 2 * NTD], FP32, tag="bqv", name="bqv")
            nc.sync.dma_start(bqv_t[:], dram["bqv"][:, :])
            bq_t = [bqv_t[:, e : e + 1] for e in range(NTD)]
            bv_t = [bqv_t[:, NTD + e : NTD + e + 1] for e in range(NTD)]

            project(psPp, Wv_t, bv_t, vT_t, store=True)

        # ---- Phase B: v natural layout via XBAR transpose (overlaps qT) ----
        with tc.tile_pool(name="vp", bufs=1) as vp:
            v_t = [
                vp.tile([PT, D], BF16, tag=f"v{k}", name=f"v{k}") for k in range(NTS)
            ]
            for k in range(NTS):
                nc.sync.dma_start(
                    v_t[k][:], vT_s[:, k * PT : (k + 1) * PT], transpose=True
                )

            # Wq per-tile loads, prefetched during the vT phase
            Wq_t = []
            for k in range(NTD):
                wq = xTqp.tile([PT, D], BF16, tag=f"wq{k}", name=f"wq{k}")
                nc.sync.dma_start(wq[:], dram["WqTt"][:, k * D : (k + 1) * D])
                Wq_t.append(wq)

            # ---- Phase C: qT projection (SBUF-resident) ----
            project(psPp, Wq_t, bq_t, qT_t, store=False)
            psP_ctx.__exit__(None, None, None)

            # constants for phase D (emitted late so they don't delay xT/W loads)
            wobig = constp.tile([PT, NTD * D], BF16, tag="wobig", name="wobig")
            nc.sync.dma_start(wobig[:], dram["WoTt"][:, :])
            WoT_t = [wobig[:, d * D : (d + 1) * D] for d in range(NTD)]
            bo_t = constp.tile([PT, D], BF16, tag="bo", name="bo")
            nc.sync.dma_start(bo_t[:], dram["bob"][:, :])
            onut_t = constp.tile([PT, 1 + PT], BF16, tag="onut", name="onut")
            nc.sync.dma_start(onut_t[:], dram["onut"][:, :])
            ones_t = onut_t[:, 0:1]
            utri_t = onut_t[:, 1 : 1 + PT]

            # ---- Phase D: attention q-blocks ----
            with (
                tc.tile_pool(name="eTp", bufs=1) as eTp,
                tc.tile_pool(name="ctxp", bufs=1) as ctxp,
                tc.tile_pool(name="rlp", bufs=1) as rlp,
                tc.tile_pool(name="outp", bufs=2) as outp,
                tc.tile_pool(name="psS", bufs=2, space="PSUM") as psSp,
                tc.tile_pool(name="psC", bufs=2, space="PSUM") as psCp,
                tc.tile_pool(name="psL", bufs=2, space="PSUM") as psLp,
                tc.tile_pool(name="psO", bufs=2, space="PSUM") as psOp,
            ):
                for c in range(NBLK):
                    q0 = c * QB
                    kmax = KPB * (c + 1) if causal else NTS

                    # scoresT + exp -> eT tiles (ragged in the diagonal region)
                    eT_t = []
                    for ki in range(kmax):
                        m = ki - KPB * c  # >=0 in diagonal region
                        lo = m * PT if (causal and m > 0) else 0
                        ps = psSp.tile([PT, QB], FP32, tag="s", name="s")
                        for e in range(NTD):
                            nc.tensor.matmul(
                                ps[:, lo:QB],
                                vT_t[e][:, ki * PT : (ki + 1) * PT],
                                qT_t[e][:, q0 + lo : q0 + QB],
                                start=(e == 0),
                                stop=(e == NTD - 1),
                            )
                        et = eTp.tile([PT, QB], BF16, tag=f"e{ki}", name=f"e{ki}")
                        nc.scalar.activation(
                            et[:, lo:QB], ps[:, lo:QB], ACT.Exp, scale=float(SCALE)
                        )
                        if causal and m >= 0:
                            nc.vector.tensor_mul(
                                et[:, m * PT : (m + 1) * PT],
                                et[:, m * PT : (m + 1) * PT],
                                utri_t,
                            )
                        eT_t.append(et)

                    # ctxT[d, q-block], accumulated over k tiles (ragged on diag)
                    ctx_t = []
                    for d in range(NTD):
                        pc = psCp.tile([PT, QB], FP32, tag="c", name="c")
                        for ki in range(kmax):
                            m = ki - KPB * c
                            lo = m * PT if (causal and m > 0) else 0
                            nc.tensor.matmul(
                                pc[:, lo:QB],
                                v_t[ki][:, d * PT : (d + 1) * PT],
                                eT_t[ki][:, lo:QB],
                                start=(ki == 0),
                                stop=(ki == kmax - 1),
                            )
                        cx = ctxp.tile([PT, QB], BF16, tag=f"cx{d}", name=f"cx{d}")
                        nc.vector.tensor_copy(cx[:], pc[:])
                        ctx_t.append(cx)

                    # softmax denominators per q sub-tile: l = eT.T @ ones,
                    # accumulating only tiles whose qt columns are valid
                    rl_t = []
                    for qt in range(KPB):
                        klim = min(kmax, KPB * c + qt + 1) if causal else kmax
                        pl = psLp.tile([PT, 1], FP32, tag="l", name="l")
                        for ki in range(klim):
                            nc.tensor.matmul(
                                pl[:],
                                eT_t[ki][:, qt * PT : (qt + 1) * PT],
                                ones_t,
                                start=(ki == 0),
                                stop=(ki == klim - 1),
                            )
                        r = rlp.tile([PT, 1], FP32, tag=f"rl{qt}", name=f"rl{qt}")
                        nc.vector.reciprocal(r[:], pl[:])
                        rl_t.append(r)

                    # out projection + normalize + bias + store (bf16 DVE path)
                    for qt in range(KPB):
                        os1 = outp.tile([PT, D], BF16, tag="os1", name="os1")
                        os_ = outp.tile([PT, D], BF16, tag="os", name="os")
                        pos = [
                            psOp.tile([PT, 512], FP32, tag="o", name="o")
                            for _ in range(D // 512)
                        ]
                        for d in range(NTD):
                            for ec in range(D // 512):
                                nc.tensor.matmul(
                                    pos[ec][:],
                                    ctx_t[d][:, qt * PT : (qt + 1) * PT],
                                    WoT_t[d][:, ec * 512 : (ec + 1) * 512],
                                    start=(d == 0),
                                    stop=(d == NTD - 1),
                                )
                        # per-half scale+bias+store so the final store chain
                        # exposes ~one half-tile of latency, not a full row;
                        # the 1/l scale runs on ACT (idle here) so only the
                        # bias-add occupies DVE
                        for ec in range(D // 512):
                            nc.scalar.activation(
                                os1[:, ec * 512 : (ec + 1) * 512],
                                pos[ec][:],
                                ACT.Identity,
                                scale=rl_t[qt][:],
                            )
                            nc.vector.tensor_add(
                                os_[:, ec * 512 : (ec + 1) * 512],
                                os1[:, ec * 512 : (ec + 1) * 512],
                                bo_t[:, ec * 512 : (ec + 1) * 512],
                            )
                            nc.sync.dma_start(
                                dram["out"][
                                    q0 + qt * PT : q0 + (qt + 1) * PT,
                                    ec * 512 : (ec + 1) * 512,
                                ],
                                os_[:, ec * 512 : (ec + 1) * 512],
                            )


_TRIL = None


def _detect_causal(mask: np.ndarray) -> bool:
    global _TRIL
    m0 = np.asarray(mask[0])
    if bool(m0[0, 1]):
        if not m0.all() or not np.asarray(mask).all():
            raise NotImplementedError("unsupported mask pattern")
        return False
    if _TRIL is None:
        _TRIL = np.tril(np.ones((S, S), dtype=bool))
    for b in range(mask.shape[0]):
        if not np.array_equal(np.asarray(mask[b]), _TRIL):
            raise NotImplementedError("unsupported mask pattern")
    return True


def kernel(x, mask, Wq, bq, Wk, bk, Wv, bv, Wo, bo):
    import ml_dtypes

    x = np.asarray(x, dtype=np.float32)
    causal = _detect_causal(np.asarray(mask))
    nc = build_nc(causal)

    def tiled(WT):
        # [D, D] -> [PT, NTD*D]: block k holds WT rows [k*PT, (k+1)*PT)
        return np.ascontiguousarray(
            WT.reshape(NTD, PT, D).transpose(1, 0, 2).reshape(PT, NTD * D)
        )

    WqT = np.ascontiguousarray(np.asarray(Wq, dtype=np.float32).T).astype(
        ml_dtypes.bfloat16
    )
    WvT = np.ascontiguousarray(np.asarray(Wv, dtype=np.float32).T).astype(
        ml_dtypes.bfloat16
    )
    WoT = np.ascontiguousarray(np.asarray(Wo, dtype=np.float32).T).astype(
        ml_dtypes.bfloat16
    )
    bqv = np.concatenate(
        [
            np.asarray(bq, dtype=np.float32).reshape(NTD, PT).T,
            np.asarray(bv, dtype=np.float32).reshape(NTD, PT).T,
        ],
        axis=1,
    )
    onut = np.concatenate(
        [
            np.ones((PT, 1), dtype=np.float32),
            np.triu(np.ones((PT, PT), dtype=np.float32)),
        ],
        axis=1,
    ).astype(ml_dtypes.bfloat16)
    base = {
        "WvT": WvT,
        "WqTt": tiled(WqT),
        "WoTt": tiled(WoT),
        "bqv": np.ascontiguousarray(bqv),
        "bob": np.tile(np.asarray(bo, dtype=np.float32).reshape(1, D), (PT, 1)).astype(ml_dtypes.bfloat16),
        "onut": onut,
    }
    in_maps = [
        {"xT": np.ascontiguousarray(x[b].T).astype(ml_dtypes.bfloat16), **base}
        for b in range(B)
    ]
    res = run_bass_kernel_spmd(nc, in_maps, list(range(B)))
    out = np.stack(
        [np.asarray(res.results[i]["out"]).astype(np.float32) for i in range(B)]
    )
    return out


if __name__ == "__main__":
    rng = np.random.default_rng(0)
    x = rng.standard_normal((B, S, D), dtype=np.float32)
    mask = np.broadcast_to(np.tril(np.ones((S, S), dtype=bool)), (B, S, S))
    mk = lambda *s: (rng.standard_normal(s, dtype=np.float32) * 0.02)
    out = kernel(
        x, mask, mk(D, D), mk(D), mk(D, D), mk(D), mk(D, D), mk(D), mk(D, D), mk(D)
    )
    print(out.shape, out.dtype)
